# revision 4
# baseline (speedup 1.0000x reference)
"""CDSNetwork (moe_routing) Trainium2 kernel.

Strategy: expert-parallel over the 8 agents. Host sorts the B*T*N tokens by
agent id, core `a` processes all tokens of agent `a` (padded to CAP, a
multiple of 512). Each core runs the full network for its tokens:

    shared = relu(relu(x @ We1 + be1) @ We2 + be2)        # [cap, 512]
    af     = relu(shared @ W1[a] + b1[a]) @ W2[a] + b2[a] # [cap, 256]
    comb   = [shared, af]                                 # [cap, 768]
    value  = comb @ Wv + bv                               # [cap]
    logits = relu(comb @ Wp1 + bp1) @ Wp2 + bp2           # [cap, 16]

On-chip layout is feature-major: activations live as [features(part), tokens]
tiles of [128, 512]; weights are lhsT tiles [k=128, m<=128] sliced out of one
big SBUF buffer. All matmuls run in float32r (full-rate fp32 path on TRN2).

The walrus codegen limits sync-wait slots per instruction (fp32 matmul: 1),
so a post-pass moves excess Tile-generated waits onto NoOps on the same
engine queue.
"""

import numpy as np

import concourse.bass as bass
import concourse.mybir as mybir
import concourse.tile as tile
from concourse.bass import ts
from concourse.bass_utils import run_bass_kernel_spmd

F32 = mybir.dt.float32
F32R = mybir.dt.float32r
RELU = mybir.ActivationFunctionType.Relu
IDENT = mybir.ActivationFunctionType.Identity
ADD = mybir.AluOpType.add
MAX = mybir.AluOpType.max

OBS = 128
ACT_DIM = 16
N_AGENTS = 8
HID = 512
ASD = 256
ASH = 256
B, T = 32, 256
NTOK = B * T * N_AGENTS

# weight buffer column offsets (all tiles are [128, cols])
C_WE1 = 0            # 4 m-tiles of 128           -> 512
C_WE2 = 512          # 4 k-tiles x [128, 512]     -> 2048
C_W1 = 2560          # 4 k-tiles x [128, 256]     -> 1024
C_W2 = 3584          # 2 k-tiles x [128, 256]     -> 512
C_WV = 4096          # 6 k-tiles x [128, 1]       -> 6
C_WP1 = 4102         # 6 k-tiles x [128, 512]     -> 3072
C_WP2 = 7174         # 4 k-tiles x [128, 16]      -> 64
WCOLS = 7238

# bias buffer columns
BC_E1, BC_E2, BC_A1, BC_A2, BC_P1, BC_P2, BC_V = 0, 4, 8, 10, 12, 16, 17
BCOLS = 18


def split_excess_waits(nc, limit=1):
    """Move excess sync waits onto same-engine NoOps (walrus slot limits)."""
    for fn in nc.m.functions:
        for blk in fn.blocks:
            insts = blk.instructions
            i = 0
            while i < len(insts):
                inst = insts[i]
                si = getattr(inst, "sync_info", None)
                if si is not None and len(si.on_wait) > limit:
                    waits = list(si.on_wait)
                    excess, keep = waits[:-limit], waits[-limit:]
                    si.on_wait = keep
                    nops = [
                        mybir.InstNoOp(
                            name=f"{inst.name}_wsplit{j}",
                            sync_info=mybir.SyncInfo(on_wait=[w], on_update=[]),
                            bass_nofuse=True,
                            engine=inst.engine,
                        )
                        for j, w in enumerate(excess)
                    ]
                    insts[i:i] = nops
                    i += len(nops)
                i += 1


def build_program(cap, repeat=1):
    """Build the SPMD Bass program for `cap` tokens per core."""
    tt = cap // 512
    nc = bass.Bass(trn_type="TRN2", debug=False, num_devices=N_AGENTS)

    xt_d = nc.dram_tensor("xt", [128, cap], F32R, kind="ExternalInput").ap()
    wts_d = nc.dram_tensor("wts", [128, WCOLS], F32R, kind="ExternalInput").ap()
    bias_d = nc.dram_tensor("bias", [128, BCOLS], F32, kind="ExternalInput").ap()
    out_d = nc.dram_tensor("out", [17, cap], F32, kind="ExternalOutput").ap()

    with tile.TileContext(nc) as tc:
        with (
            tc.tile_pool(name="wpool", bufs=1) as wpool,
            tc.tile_pool(name="xpool", bufs=3) as xpool,
            tc.tile_pool(name="actp", bufs=2) as actp,
            tc.tile_pool(name="obuf", bufs=1) as obuf,
            tc.tile_pool(name="pmm", bufs=5, space="PSUM") as pmm,
            tc.tile_pool(name="pv", bufs=1, space="PSUM") as pv,
            tc.tile_pool(name="pp2", bufs=2, space="PSUM") as pp2,
        ):
            wt = wpool.tile([128, WCOLS], F32R)
            nc.sync.dma_start(wt[:], wts_d[:])
            bt = wpool.tile([128, BCOLS], F32)
            nc.sync.dma_start(bt[:], bias_d[:])
            ob = obuf.tile([16, cap], F32)
            vob = obuf.tile([1, cap], F32)

            def w_e1(m):
                return wt[:, C_WE1 + m * 128 : C_WE1 + (m + 1) * 128]

            def w_e2(k, m):
                c = C_WE2 + k * 512 + m * 128
                return wt[:, c : c + 128]

            def w_a1(k, m):
                c = C_W1 + k * 256 + m * 128
                return wt[:, c : c + 128]

            def w_a2(k, m):
                c = C_W2 + k * 256 + m * 128
                return wt[:, c : c + 128]

            def w_v(k):
                return wt[:, C_WV + k : C_WV + k + 1]

            def w_p1(k, m):
                c = C_WP1 + k * 512 + m * 128
                return wt[:, c : c + 128]

            def w_p2(k):
                return wt[:, C_WP2 + k * 16 : C_WP2 + (k + 1) * 16]

            def act_drain(dst, ps, func, bias_ap):
                nc.scalar.activation(dst, ps, func, bias=bias_ap)

            def dve_drain(dst, ps, relu, bias_ap):
                if relu:
                    nc.vector.tensor_scalar(dst, ps, bias_ap, 0.0, ADD, MAX)
                else:
                    nc.vector.tensor_scalar(dst, ps, bias_ap, None, ADD)

            for _ in range(repeat):
                for t in range(tt):
                    x = xpool.tile([128, 512], F32R, tag="x")
                    nc.sync.dma_start(x[:], xt_d[:, ts(t, 512)])

                    # shared encoder layer 1: [128 obs] -> [512], relu (ACT)
                    s1 = []
                    for m in range(4):
                        ps = pmm.tile([128, 512], F32, tag="ps")
                        nc.tensor.matmul(ps[:], w_e1(m), x[:], start=True, stop=True)
                        o = actp.tile([128, 512], F32R, tag=f"s1_{m}")
                        act_drain(o[:], ps[:], RELU, bt[:, BC_E1 + m : BC_E1 + m + 1])
                        s1.append(o)

                    # shared encoder layer 2: [512] -> [512], relu (DVE)
                    s2 = []
                    for m in range(4):
                        ps = pmm.tile([128, 512], F32, tag="ps")
                        for k in range(4):
                            nc.tensor.matmul(
                                ps[:], w_e2(k, m), s1[k][:],
                                start=(k == 0), stop=(k == 3),
                            )
                        o = actp.tile([128, 512], F32R, tag=f"s2_{m}")
                        dve_drain(o[:], ps[:], True, bt[:, BC_E2 + m : BC_E2 + m + 1])
                        s2.append(o)

                    # agent mlp layer 1: [512] -> [256], relu (ACT)
                    h1 = []
                    for m in range(2):
                        ps = pmm.tile([128, 512], F32, tag="ps")
                        for k in range(4):
                            nc.tensor.matmul(
                                ps[:], w_a1(k, m), s2[k][:],
                                start=(k == 0), stop=(k == 3),
                            )
                        o = actp.tile([128, 512], F32R, tag=f"h1_{m}")
                        act_drain(o[:], ps[:], RELU, bt[:, BC_A1 + m : BC_A1 + m + 1])
                        h1.append(o)

                    # agent mlp layer 2: [256] -> [256], no relu (DVE)
                    af = []
                    for m in range(2):
                        ps = pmm.tile([128, 512], F32, tag="ps")
                        for k in range(2):
                            nc.tensor.matmul(
                                ps[:], w_a2(k, m), h1[k][:],
                                start=(k == 0), stop=(k == 1),
                            )
                        o = actp.tile([128, 512], F32R, tag=f"af_{m}")
                        dve_drain(o[:], ps[:], False, bt[:, BC_A2 + m : BC_A2 + m + 1])
                        af.append(o)

                    comb = s2 + af  # 6 k-tiles of [128, 512]

                    # value head: [768] -> [1] (ACT)
                    psv = pv.tile([1, 512], F32, tag="psv")
                    for k in range(6):
                        nc.tensor.matmul(
                            psv[:], w_v(k), comb[k][:],
                            start=(k == 0), stop=(k == 5),
                        )
                    act_drain(
                        vob[0:1, ts(t, 512)], psv[:], IDENT,
                        bt[0:1, BC_V : BC_V + 1],
                    )

                    # policy layer 1: [768] -> [512], relu (DVE)
                    q = []
                    for m in range(4):
                        ps = pmm.tile([128, 512], F32, tag="ps")
                        for k in range(6):
                            nc.tensor.matmul(
                                ps[:], w_p1(k, m), comb[k][:],
                                start=(k == 0), stop=(k == 5),
                            )
                        o = actp.tile([128, 512], F32R, tag=f"q_{m}")
                        dve_drain(o[:], ps[:], True, bt[:, BC_P1 + m : BC_P1 + m + 1])
                        q.append(o)

                    # policy layer 2: [512] -> [16] (ACT)
                    ps16 = pp2.tile([16, 512], F32, tag="ps16")
                    for k in range(4):
                        nc.tensor.matmul(
                            ps16[:], w_p2(k), q[k][:],
                            start=(k == 0), stop=(k == 3),
                        )
                    act_drain(
                        ob[0:16, ts(t, 512)], ps16[:], IDENT,
                        bt[0:16, BC_P2 : BC_P2 + 1],
                    )

            nc.sync.dma_start(out_d[0:16, :], ob[:])
            nc.sync.dma_start(out_d[16:17, :], vob[:])

    split_excess_waits(nc)
    return nc


_PROGRAM_CACHE = {}


def _get_program(cap, repeat=1):
    key = (cap, repeat)
    if key not in _PROGRAM_CACHE:
        _PROGRAM_CACHE[key] = build_program(cap, repeat)
    return _PROGRAM_CACHE[key]


def _ktiles(w):
    """[K, M] weight -> [128, (K//128)*M]: k-tiles concatenated column-wise."""
    k, m = w.shape
    return np.ascontiguousarray(
        w.reshape(k // 128, 128, m).transpose(1, 0, 2).reshape(128, -1)
    )


def _build_in_maps(obs, agent_ids, We1, be1, We2, be2, W1, b1, W2, b2,
                   Wv, bv, Wp1, bp1, Wp2, bp2):
    x_flat = np.ascontiguousarray(np.asarray(obs, dtype=np.float32)).reshape(NTOK, OBS)
    ids = np.asarray(agent_ids).reshape(-1)
    order = np.argsort(ids, kind="stable")
    counts = np.bincount(ids, minlength=N_AGENTS)
    assert counts.sum() == NTOK
    starts = np.zeros(N_AGENTS + 1, np.int64)
    starts[1:] = np.cumsum(counts)
    cap = max(512, int(-(-counts.max() // 512) * 512))

    shared_cols = np.empty((128, WCOLS), np.float32)
    shared_cols[:, C_WE1:C_WE1 + 512] = np.asarray(We1, np.float32)
    shared_cols[:, C_WE2:C_WE2 + 2048] = _ktiles(np.asarray(We2, np.float32))
    shared_cols[:, C_WV:C_WV + 6] = _ktiles(np.asarray(Wv, np.float32))
    shared_cols[:, C_WP1:C_WP1 + 3072] = _ktiles(np.asarray(Wp1, np.float32))
    shared_cols[:, C_WP2:C_WP2 + 64] = _ktiles(np.asarray(Wp2, np.float32))

    bias = np.zeros((128, BCOLS), np.float32)
    bias[:, BC_E1:BC_E1 + 4] = np.asarray(be1, np.float32).reshape(4, 128).T
    bias[:, BC_E2:BC_E2 + 4] = np.asarray(be2, np.float32).reshape(4, 128).T
    bias[:, BC_P1:BC_P1 + 4] = np.asarray(bp1, np.float32).reshape(4, 128).T
    bias[:16, BC_P2] = np.asarray(bp2, np.float32)
    bias[0, BC_V] = np.float32(np.asarray(bv, np.float32).reshape(-1)[0])

    in_maps, idx_per_core = [], []
    for a in range(N_AGENTS):
        idx = order[starts[a]:starts[a + 1]]
        idx_per_core.append(idx)
        xa = np.zeros((cap, OBS), np.float32)
        xa[:len(idx)] = x_flat[idx]
        xt = np.ascontiguousarray(xa.T)

        wts = shared_cols.copy()
        wts[:, C_W1:C_W1 + 1024] = _ktiles(np.asarray(W1[a], np.float32))
        wts[:, C_W2:C_W2 + 512] = _ktiles(np.asarray(W2[a], np.float32))

        ba = bias.copy()
        ba[:, BC_A1:BC_A1 + 2] = np.asarray(b1[a], np.float32).reshape(2, 128).T
        ba[:, BC_A2:BC_A2 + 2] = np.asarray(b2[a], np.float32).reshape(2, 128).T

        in_maps.append({"xt": xt, "wts": wts, "bias": ba})
    return in_maps, idx_per_core, cap


def run(inputs, repeat=1):
    """Run on HW; returns (values, logits) plus the raw results."""
    in_maps, idx_per_core, cap = _build_in_maps(**inputs)
    nc = _get_program(cap, repeat)
    res = run_bass_kernel_spmd(nc, in_maps, core_ids=list(range(N_AGENTS)))

    values_flat = np.zeros(NTOK, np.float32)
    logits_flat = np.zeros((NTOK, ACT_DIM), np.float32)
    for a in range(N_AGENTS):
        idx = idx_per_core[a]
        out = res.results[a]["out"]
        values_flat[idx] = out[16, :len(idx)]
        logits_flat[idx] = out[0:ACT_DIM, :len(idx)].T
    values = values_flat.reshape(B, T, N_AGENTS)
    logits = logits_flat.reshape(B, T, N_AGENTS, ACT_DIM)
    return values, logits


def kernel(**inputs):
    return run(inputs, repeat=1)


# revision 9
# speedup vs baseline: 9901.5041x; 9901.5041x over previous
"""CDSNetwork (moe_routing) Trainium2 kernel.

Strategy: expert-parallel over the 8 agents. Host sorts the B*T*N tokens by
agent id, core `a` processes all tokens of agent `a` (padded to CAP, a
multiple of 512). Each core runs the full network for its tokens:

    shared = relu(relu(x @ We1 + be1) @ We2 + be2)        # [cap, 512]
    af     = relu(shared @ W1[a] + b1[a]) @ W2[a] + b2[a] # [cap, 256]
    comb   = [shared, af]                                 # [cap, 768]
    value  = comb @ Wv + bv                               # [cap]
    logits = relu(comb @ Wp1 + bp1) @ Wp2 + bp2           # [cap, 16]

On-chip layout is feature-major: activations live as [features(part), tokens]
tiles of [128, 512]; weights are lhsT tiles [k=128, m<=128] sliced out of one
big SBUF buffer. All matmuls run in float32r (full-rate fp32 path on TRN2).

The walrus codegen limits sync-wait slots per instruction (fp32 matmul: 1),
so a post-pass moves excess Tile-generated waits onto NoOps on the same
engine queue.
"""

import numpy as np

import concourse.bass as bass
import concourse.mybir as mybir
import concourse.tile as tile
from concourse.bass import ts

F32 = mybir.dt.float32
F32R = mybir.dt.float32r
RELU = mybir.ActivationFunctionType.Relu
IDENT = mybir.ActivationFunctionType.Identity
ADD = mybir.AluOpType.add
MAX = mybir.AluOpType.max

OBS = 128
ACT_DIM = 16
N_AGENTS = 8
HID = 512
ASD = 256
ASH = 256
B, T = 32, 256
NTOK = B * T * N_AGENTS

# weight buffer column offsets (all tiles are [128, cols])
C_WE1 = 0            # 4 m-tiles of 128           -> 512
C_WE2 = 512          # 4 k-tiles x [128, 512]     -> 2048
C_W1 = 2560          # 4 k-tiles x [128, 256]     -> 1024
C_W2 = 3584          # 2 k-tiles x [128, 256]     -> 512
C_WV = 4096          # 6 k-tiles x [128, 1]       -> 6
C_WP1 = 4102         # 6 k-tiles x [128, 512]     -> 3072
C_WP2 = 7174         # 4 k-tiles x [128, 16]      -> 64
WCOLS = 7238

# bias buffer columns
BC_E1, BC_E2, BC_A1, BC_A2, BC_P1, BC_P2, BC_V = 0, 4, 8, 10, 12, 16, 17
BCOLS = 18


def split_excess_waits(nc, limit=1):
    """Move excess sync waits onto same-engine NoOps (walrus slot limits)."""
    for fn in nc.m.functions:
        for blk in fn.blocks:
            insts = blk.instructions
            i = 0
            while i < len(insts):
                inst = insts[i]
                si = getattr(inst, "sync_info", None)
                if si is not None and len(si.on_wait) > limit:
                    waits = list(si.on_wait)
                    excess, keep = waits[:-limit], waits[-limit:]
                    si.on_wait = keep
                    nops = [
                        mybir.InstNoOp(
                            name=f"{inst.name}_wsplit{j}",
                            sync_info=mybir.SyncInfo(on_wait=[w], on_update=[]),
                            bass_nofuse=True,
                            engine=inst.engine,
                        )
                        for j, w in enumerate(excess)
                    ]
                    insts[i:i] = nops
                    i += len(nops)
                i += 1


def build_program(cap, repeat=1):
    """Build the SPMD Bass program for `cap` tokens per core."""
    tt = cap // 512
    nc = bass.Bass(trn_type="TRN2", debug=False, num_devices=N_AGENTS)

    xt_d = nc.dram_tensor("xt", [128, cap], F32R, kind="ExternalInput").ap()
    wts_d = nc.dram_tensor("wts", [128, WCOLS], F32R, kind="ExternalInput").ap()
    bias_d = nc.dram_tensor("bias", [128, BCOLS], F32, kind="ExternalInput").ap()
    out_d = nc.dram_tensor("out", [17, cap], F32, kind="ExternalOutput").ap()

    with tile.TileContext(nc) as tc:
        with (
            tc.tile_pool(name="wpool", bufs=1) as wpool,
            tc.tile_pool(name="xpool", bufs=3) as xpool,
            tc.tile_pool(name="actp", bufs=2) as actp,
            tc.tile_pool(name="obuf", bufs=1) as obuf,
            tc.tile_pool(name="pmm", bufs=5, space="PSUM") as pmm,
            tc.tile_pool(name="pv", bufs=1, space="PSUM") as pv,
            tc.tile_pool(name="pp2", bufs=2, space="PSUM") as pp2,
        ):
            wt = wpool.tile([128, WCOLS], F32R)
            nc.sync.dma_start(wt[:], wts_d[:])
            bt = wpool.tile([128, BCOLS], F32)
            nc.sync.dma_start(bt[:], bias_d[:])
            ob = obuf.tile([16, cap], F32)
            vob = obuf.tile([1, cap], F32)

            def w_e1(m):
                return wt[:, C_WE1 + m * 128 : C_WE1 + (m + 1) * 128]

            def w_e2(k, m):
                c = C_WE2 + k * 512 + m * 128
                return wt[:, c : c + 128]

            def w_a1(k, m):
                c = C_W1 + k * 256 + m * 128
                return wt[:, c : c + 128]

            def w_a2(k, m):
                c = C_W2 + k * 256 + m * 128
                return wt[:, c : c + 128]

            def w_v(k):
                return wt[:, C_WV + k : C_WV + k + 1]

            def w_p1(k, m):
                c = C_WP1 + k * 512 + m * 128
                return wt[:, c : c + 128]

            def w_p2(k):
                return wt[:, C_WP2 + k * 16 : C_WP2 + (k + 1) * 16]

            def act_drain(dst, ps, func, bias_ap):
                nc.scalar.activation(dst, ps, func, bias=bias_ap)

            def dve_drain(dst, ps, relu, bias_ap):
                if relu:
                    nc.vector.tensor_scalar(dst, ps, bias_ap, 0.0, ADD, MAX)
                else:
                    nc.vector.tensor_scalar(dst, ps, bias_ap, None, ADD)

            for _ in range(repeat):
                for t in range(tt):
                    x = xpool.tile([128, 512], F32R, tag="x")
                    nc.sync.dma_start(x[:], xt_d[:, ts(t, 512)])

                    # shared encoder layer 1: [128 obs] -> [512], relu (ACT)
                    s1 = []
                    for m in range(4):
                        ps = pmm.tile([128, 512], F32, tag="ps")
                        nc.tensor.matmul(ps[:], w_e1(m), x[:], start=True, stop=True)
                        o = actp.tile([128, 512], F32R, tag=f"s1_{m}")
                        act_drain(o[:], ps[:], RELU, bt[:, BC_E1 + m : BC_E1 + m + 1])
                        s1.append(o)

                    # shared encoder layer 2: [512] -> [512], relu (DVE)
                    s2 = []
                    for m in range(4):
                        ps = pmm.tile([128, 512], F32, tag="ps")
                        for k in range(4):
                            nc.tensor.matmul(
                                ps[:], w_e2(k, m), s1[k][:],
                                start=(k == 0), stop=(k == 3),
                            )
                        o = actp.tile([128, 512], F32R, tag=f"s2_{m}")
                        dve_drain(o[:], ps[:], True, bt[:, BC_E2 + m : BC_E2 + m + 1])
                        s2.append(o)

                    # agent mlp layer 1: [512] -> [256], relu (ACT)
                    h1 = []
                    for m in range(2):
                        ps = pmm.tile([128, 512], F32, tag="ps")
                        for k in range(4):
                            nc.tensor.matmul(
                                ps[:], w_a1(k, m), s2[k][:],
                                start=(k == 0), stop=(k == 3),
                            )
                        o = actp.tile([128, 512], F32R, tag=f"h1_{m}")
                        act_drain(o[:], ps[:], RELU, bt[:, BC_A1 + m : BC_A1 + m + 1])
                        h1.append(o)

                    # agent mlp layer 2: [256] -> [256], no relu (DVE)
                    af = []
                    for m in range(2):
                        ps = pmm.tile([128, 512], F32, tag="ps")
                        for k in range(2):
                            nc.tensor.matmul(
                                ps[:], w_a2(k, m), h1[k][:],
                                start=(k == 0), stop=(k == 1),
                            )
                        o = actp.tile([128, 512], F32R, tag=f"af_{m}")
                        dve_drain(o[:], ps[:], False, bt[:, BC_A2 + m : BC_A2 + m + 1])
                        af.append(o)

                    comb = s2 + af  # 6 k-tiles of [128, 512]

                    # value head: [768] -> [1] (ACT)
                    psv = pv.tile([1, 512], F32, tag="psv")
                    for k in range(6):
                        nc.tensor.matmul(
                            psv[:], w_v(k), comb[k][:],
                            start=(k == 0), stop=(k == 5),
                        )
                    act_drain(
                        vob[0:1, ts(t, 512)], psv[:], IDENT,
                        bt[0:1, BC_V : BC_V + 1],
                    )

                    # policy layer 1: [768] -> [512], relu (DVE)
                    q = []
                    for m in range(4):
                        ps = pmm.tile([128, 512], F32, tag="ps")
                        for k in range(6):
                            nc.tensor.matmul(
                                ps[:], w_p1(k, m), comb[k][:],
                                start=(k == 0), stop=(k == 5),
                            )
                        o = actp.tile([128, 512], F32R, tag=f"q_{m}")
                        dve_drain(o[:], ps[:], True, bt[:, BC_P1 + m : BC_P1 + m + 1])
                        q.append(o)

                    # policy layer 2: [512] -> [16] (ACT)
                    ps16 = pp2.tile([16, 512], F32, tag="ps16")
                    for k in range(4):
                        nc.tensor.matmul(
                            ps16[:], w_p2(k), q[k][:],
                            start=(k == 0), stop=(k == 3),
                        )
                    act_drain(
                        ob[0:16, ts(t, 512)], ps16[:], IDENT,
                        bt[0:16, BC_P2 : BC_P2 + 1],
                    )

            nc.sync.dma_start(out_d[0:16, :], ob[:])
            nc.sync.dma_start(out_d[16:17, :], vob[:])

    split_excess_waits(nc)
    return nc


class _Runner:
    """SPMD executor with a cached jitted callable (one lowering+compile,
    then cheap repeated calls). Mirrors bass2jax.run_bass_via_pjrt."""

    def __init__(self, nc, n_cores):
        import jax
        from jax.experimental.shard_map import shard_map
        from jax.sharding import Mesh, PartitionSpec
        from concourse import bass2jax, mybir as _mybir

        bass2jax.install_neuronx_cc_hook()
        self.n_cores = n_cores
        partition_name = (
            nc.partition_id_tensor.name if nc.partition_id_tensor else None
        )
        in_names, out_names, out_avals, zero_shapes = [], [], [], []
        for alloc in nc.m.functions[0].allocations:
            if not isinstance(alloc, _mybir.MemoryLocationSet):
                continue
            name = alloc.memorylocations[0].name
            if alloc.kind == "ExternalInput":
                if name != partition_name:
                    in_names.append(name)
            elif alloc.kind == "ExternalOutput":
                shape = tuple(alloc.tensor_shape)
                dtype = _mybir.dt.np(alloc.dtype)
                out_names.append(name)
                out_avals.append(jax.core.ShapedArray(shape, dtype))
                zero_shapes.append((shape, dtype))
        self.in_names = list(in_names)
        self.out_names = out_names
        self.out_avals = out_avals
        self.zero_shapes = zero_shapes
        n_params = len(in_names)
        all_in_names = in_names + out_names
        if partition_name is not None:
            all_in_names.append(partition_name)
        donate = tuple(range(n_params, n_params + len(out_names)))

        def _body(*args):
            operands = list(args)
            if partition_name is not None:
                operands.append(bass2jax.partition_id_tensor())
            outs = bass2jax._bass_exec_p.bind(
                *operands,
                out_avals=tuple(out_avals),
                in_names=tuple(all_in_names),
                out_names=tuple(out_names),
                lowering_input_output_aliases=(),
                sim_require_finite=True,
                sim_require_nnan=True,
                nc=nc,
            )
            return tuple(outs)

        devices = jax.devices()[:n_cores]
        mesh = Mesh(np.asarray(devices), ("core",))
        self.mesh = mesh
        nspecs = n_params + len(out_names)
        self._fn = jax.jit(
            shard_map(
                _body,
                mesh=mesh,
                in_specs=(PartitionSpec("core"),) * nspecs,
                out_specs=(PartitionSpec("core"),) * len(out_names),
                check_rep=False,
            ),
            donate_argnums=donate,
            keep_unused=True,
        )

    def prepare(self, in_maps):
        """Stage inputs on-device once for repeated timing calls."""
        import jax
        from jax.sharding import NamedSharding, PartitionSpec

        sh = NamedSharding(self.mesh, PartitionSpec("core"))
        concat_in = [
            np.concatenate([np.asarray(m[name]) for m in in_maps], axis=0)
            for name in self.in_names
        ]
        return [jax.device_put(c, sh) for c in concat_in]

    def call_prepared(self, dev_in, block=True):
        import jax

        n = self.n_cores
        concat_zeros = [
            np.zeros((n * s[0], *s[1:]), dt) for (s, dt) in self.zero_shapes
        ]
        outs = self._fn(*dev_in, *concat_zeros)
        if block:
            jax.block_until_ready(outs)
        return outs

    def __call__(self, in_maps):
        outs = self.call_prepared(self.prepare(in_maps), block=False)
        outs = [np.asarray(o) for o in outs]
        n = self.n_cores
        return [
            {
                name: outs[i].reshape(n, *self.out_avals[i].shape)[c]
                for i, name in enumerate(self.out_names)
            }
            for c in range(n)
        ]


_PROGRAM_CACHE = {}


def _get_runner(cap, repeat=1):
    key = (cap, repeat)
    if key not in _PROGRAM_CACHE:
        nc = build_program(cap, repeat)
        _PROGRAM_CACHE[key] = _Runner(nc, N_AGENTS)
    return _PROGRAM_CACHE[key]


def _ktiles(w):
    """[K, M] weight -> [128, (K//128)*M]: k-tiles concatenated column-wise."""
    k, m = w.shape
    return np.ascontiguousarray(
        w.reshape(k // 128, 128, m).transpose(1, 0, 2).reshape(128, -1)
    )


def _build_in_maps(obs, agent_ids, We1, be1, We2, be2, W1, b1, W2, b2,
                   Wv, bv, Wp1, bp1, Wp2, bp2):
    x_flat = np.ascontiguousarray(np.asarray(obs, dtype=np.float32)).reshape(NTOK, OBS)
    ids = np.asarray(agent_ids).reshape(-1)
    order = np.argsort(ids, kind="stable")
    counts = np.bincount(ids, minlength=N_AGENTS)
    assert counts.sum() == NTOK
    starts = np.zeros(N_AGENTS + 1, np.int64)
    starts[1:] = np.cumsum(counts)
    cap = max(512, int(-(-counts.max() // 512) * 512))

    shared_cols = np.empty((128, WCOLS), np.float32)
    shared_cols[:, C_WE1:C_WE1 + 512] = np.asarray(We1, np.float32)
    shared_cols[:, C_WE2:C_WE2 + 2048] = _ktiles(np.asarray(We2, np.float32))
    shared_cols[:, C_WV:C_WV + 6] = _ktiles(np.asarray(Wv, np.float32))
    shared_cols[:, C_WP1:C_WP1 + 3072] = _ktiles(np.asarray(Wp1, np.float32))
    shared_cols[:, C_WP2:C_WP2 + 64] = _ktiles(np.asarray(Wp2, np.float32))

    bias = np.zeros((128, BCOLS), np.float32)
    bias[:, BC_E1:BC_E1 + 4] = np.asarray(be1, np.float32).reshape(4, 128).T
    bias[:, BC_E2:BC_E2 + 4] = np.asarray(be2, np.float32).reshape(4, 128).T
    bias[:, BC_P1:BC_P1 + 4] = np.asarray(bp1, np.float32).reshape(4, 128).T
    bias[:16, BC_P2] = np.asarray(bp2, np.float32)
    bias[0, BC_V] = np.float32(np.asarray(bv, np.float32).reshape(-1)[0])

    in_maps, idx_per_core = [], []
    for a in range(N_AGENTS):
        idx = order[starts[a]:starts[a + 1]]
        idx_per_core.append(idx)
        xa = np.zeros((cap, OBS), np.float32)
        xa[:len(idx)] = x_flat[idx]
        xt = np.ascontiguousarray(xa.T)

        wts = shared_cols.copy()
        wts[:, C_W1:C_W1 + 1024] = _ktiles(np.asarray(W1[a], np.float32))
        wts[:, C_W2:C_W2 + 512] = _ktiles(np.asarray(W2[a], np.float32))

        ba = bias.copy()
        ba[:, BC_A1:BC_A1 + 2] = np.asarray(b1[a], np.float32).reshape(2, 128).T
        ba[:, BC_A2:BC_A2 + 2] = np.asarray(b2[a], np.float32).reshape(2, 128).T

        in_maps.append({"xt": xt, "wts": wts, "bias": ba})
    return in_maps, idx_per_core, cap


def run(inputs, repeat=1):
    """Run on HW; returns (values, logits)."""
    in_maps, idx_per_core, cap = _build_in_maps(**inputs)
    runner = _get_runner(cap, repeat)
    results = runner(in_maps)

    values_flat = np.zeros(NTOK, np.float32)
    logits_flat = np.zeros((NTOK, ACT_DIM), np.float32)
    for a in range(N_AGENTS):
        idx = idx_per_core[a]
        out = results[a]["out"]
        values_flat[idx] = out[16, :len(idx)]
        logits_flat[idx] = out[0:ACT_DIM, :len(idx)].T
    values = values_flat.reshape(B, T, N_AGENTS)
    logits = logits_flat.reshape(B, T, N_AGENTS, ACT_DIM)
    return values, logits


def kernel(**inputs):
    return run(inputs, repeat=1)
